# revision 1
# baseline (speedup 1.0000x reference)
"""GCNFast Trainium2 kernel.

out[b] = relu(A @ x_b + GCB),  A = relu(AA_mask * GCW)  [4096, 4096]
x_b = transpose(h[b]) reshaped [Nt*Nc, d_h];  out reshaped to [bs, Ns, Nt, d_h].

Sharding over 8 cores: 4-way row-shard of A/GCB (1024 rows each) x 2-way
batch split (8 batches each). Each core computes its slice of A on-chip
(DVE masked-relu mul -> PE transpose to contraction-major), keeps the bf16
activations X [4096, 8*128] resident in SBUF, and accumulates bf16 matmuls
into PSUM with a DVE bias-add + ACT relu epilogue. bf16 operands keep the
relative error ~2e-3 (inputs quantized once; accumulation in fp32 PSUM).

Two compiled variants, selected at runtime:
 - compact: AA_mask is tile(AA, (Nt, Nt)) (what setup_inputs produces), so
   only a [128, Nc] per-m-tile mask is loaded and broadcast along t. That
   drops per-core HBM reads from ~50MB to ~34MB. Scheduling: a "triangle"
   of the first 4 m-tiles accumulates both batch halves against X tiles as
   they stream in (8 one-bank PSUM accumulators; the 2 transpose-staging
   banks are handed over exactly when the 4th pair allocates), then the
   remaining 4 m-tiles run as a PE-bound sequential pipeline fed by
   trailing gcw loads.
 - full: general AA_mask fallback (full mask shard streamed, simple
   m-tile pipeline).

Index conventions inside a core (both are pure permutations absorbed by the
on-chip transpose stage, chosen so every DMA access pattern collapses to
<=3 dims with a contiguous partition merge):
 - contraction k' = c*Nt + t  (c-major), so h's (c t) merges contiguously;
 - output row m' = s*Tsh + t  (s-major), so out's (s t) merges contiguously.
"""

from contextlib import ExitStack

import numpy as np

import concourse.mybir as mybir
import concourse.tile as tile
from concourse import bacc, masks
from concourse.bass_utils import run_bass_kernel_spmd

# Problem constants (hardcoded per harness contract).
NC_, NS, NT, DH, BS = 64, 64, 64, 128, 16
K = NC_ * NT          # 4096 contraction dim
M = NS * NT           # 4096 output rows
P_ROW, P_BATCH = 4, 2  # 4-way row shard x 2-way batch shard = 8 cores
M_SH = M // P_ROW     # 1024 rows per core
B_SH = BS // P_BATCH  # 8 batches per core
NFREE = B_SH * DH     # 1024 = moving free dim (b, d)
KT = K // 128         # 32 k-tiles
MT = M_SH // 128      # 8 m-tiles per core
T_SH = M_SH // NS     # 16 t-values per core
S_PT = 128 // T_SH    # 8 s-values per m'-tile

F32 = mybir.dt.float32
BF16 = mybir.dt.bfloat16

_cached = {}


def _build():
    nc = bacc.Bacc(
        "TRN2",
        target_bir_lowering=False,
        debug=False,
        enable_asserts=False,
        num_devices=8,
        num_swdge_queues=2,
    )

    gcw = nc.dram_tensor("gcw", [M_SH, K], F32, kind="ExternalInput").ap()
    aa = nc.dram_tensor("aa", [M_SH, K], F32, kind="ExternalInput").ap()
    gcb = nc.dram_tensor("gcb", [M_SH, DH], F32, kind="ExternalInput").ap()
    h = nc.dram_tensor("h", [B_SH, NC_, NT, DH], F32, kind="ExternalInput").ap()
    out = nc.dram_tensor("out", [B_SH, NS, T_SH, DH], F32, kind="ExternalOutput").ap()

    # row-permuted views: m' = s*T_SH + t  (s-major)
    gcw_p = gcw.rearrange("(t s) k -> s t k", t=T_SH)
    aa_p = aa.rearrange("(t s) k -> s t k", t=T_SH)
    gcb_p = gcb.rearrange("(t s) d -> s t d", t=T_SH)

    with tile.TileContext(nc) as tc:
        with ExitStack() as ctx:
            ident_pool = ctx.enter_context(tc.tile_pool(name="ident", bufs=1))
            x_pool = ctx.enter_context(tc.tile_pool(name="x", bufs=KT))
            gw_pool = ctx.enter_context(tc.tile_pool(name="gw", bufs=4))
            aa_pool = ctx.enter_context(tc.tile_pool(name="aam", bufs=4))
            am_pool = ctx.enter_context(tc.tile_pool(name="am", bufs=2))
            at_pool = ctx.enter_context(tc.tile_pool(name="at", bufs=2))
            gcb_pool = ctx.enter_context(tc.tile_pool(name="gcb", bufs=MT))
            out_pool = ctx.enter_context(tc.tile_pool(name="out", bufs=2))
            ptr_pool = ctx.enter_context(
                tc.tile_pool(name="ptr", bufs=2, space="PSUM")
            )
            pmm_pool = ctx.enter_context(
                tc.tile_pool(name="pmm", bufs=2, space="PSUM")
            )

            ident = ident_pool.tile([128, 128], BF16)
            masks.make_identity(nc, ident[:])

            # Interleave the A-stream prefetch (per-m-tile critical path
            # feeder) with the resident X tiles so neither starves: queue
            # order on the SWDGE ring follows program order.
            gw_tiles, aa_tiles, gcb_tiles, x_tiles = [], [], [], []
            for mt in range(MT):
                srows = slice(S_PT * mt, S_PT * (mt + 1))
                gw_t = gw_pool.tile([128, K], BF16)
                nc.gpsimd.dma_start(out=gw_t[:], in_=gcw_p[srows])
                aa_t = aa_pool.tile([128, K], BF16)
                nc.gpsimd.dma_start(out=aa_t[:], in_=aa_p[srows])
                gw_tiles.append(gw_t)
                aa_tiles.append(aa_t)
                # X[k'-tile] = [128 (c,t), 1024 (b,d)], cast f32->bf16 in
                # the SWDGE DMA datapath; 4 per m-tile covers all 32.
                for j in range(4 * mt, 4 * mt + 4):
                    xt = x_pool.tile([128, NFREE], BF16)
                    src = h[:, 2 * j : 2 * j + 2, :, :].rearrange(
                        "b c t d -> (c t) b d"
                    )
                    nc.gpsimd.dma_start(out=xt[:], in_=src)
                    x_tiles.append(xt)
                if mt == 0:
                    for mt2 in range(MT):
                        srows2 = slice(S_PT * mt2, S_PT * (mt2 + 1))
                        gcb_t = gcb_pool.tile([128, DH], F32)
                        nc.sync.dma_start(out=gcb_t[:], in_=gcb_p[srows2])
                        gcb_tiles.append(gcb_t)

            for mt in range(MT):
                gw_t, aa_t = gw_tiles[mt], aa_tiles[mt]
                # masked weights with fused relu: since aa >= 0,
                # relu(gw*aa) == max(gw,0)*aa. The output AP permutes the
                # free dim from t-major k to c-major k' so the transpose and
                # matmul reads stay dense:
                # am_t[m, c*Nt + t] = max(gw[m, t*Nc+c], 0) * aa[m, t*Nc+c].
                am_t = am_pool.tile([128, K], BF16)
                nc.vector.scalar_tensor_tensor(
                    am_t[:].rearrange("m (c t) -> m t c", c=NC_),
                    gw_t[:].rearrange("m (t c) -> m t c", c=NC_),
                    0.0,
                    aa_t[:].rearrange("m (t c) -> m t c", c=NC_),
                    mybir.AluOpType.max,
                    mybir.AluOpType.mult,
                )

                # A^T for this m'-tile: 32 side-by-side [128 k', 128 m'] tiles.
                at_t = at_pool.tile([128, K], BF16)
                for g in range(KT // 8):
                    ptr = ptr_pool.tile([128, 1024], BF16)
                    for j8 in range(8):
                        j = 8 * g + j8
                        nc.tensor.transpose(
                            ptr[:, 128 * j8 : 128 * j8 + 128],
                            am_t[:, 128 * j : 128 * j + 128],
                            ident[:],
                        )
                    dstslice = at_t[:, 1024 * g : 1024 * g + 1024]
                    if g % 2 == 0:
                        nc.scalar.copy(dstslice, ptr[:])
                    else:
                        nc.vector.tensor_copy(dstslice, ptr[:])

                # 32 accumulating matmuls: psum[m', (b,d)] += A^T[k']^T @ X[k']
                pm = pmm_pool.tile([128, NFREE], F32)
                for j in range(KT):
                    for nh in range(NFREE // 512):
                        nc.tensor.matmul(
                            pm[:, 512 * nh : 512 * nh + 512],
                            at_t[:, 128 * j : 128 * j + 128],
                            x_tiles[j][:, 512 * nh : 512 * nh + 512],
                            start=(j == 0),
                            stop=(j == KT - 1),
                        )

                # epilogue: bias add (broadcast over b) + relu, then store
                o_t = out_pool.tile([128, NFREE], F32)
                bias = gcb_tiles[mt][:].unsqueeze(1).broadcast_to(
                    (128, B_SH, DH)
                )
                nc.vector.tensor_add(
                    o_t[:].rearrange("p (b d) -> p b d", b=B_SH),
                    pm[:].rearrange("p (b d) -> p b d", b=B_SH),
                    bias,
                )
                nc.scalar.activation(
                    o_t[:], o_t[:], mybir.ActivationFunctionType.Relu
                )

                srows = slice(S_PT * mt, S_PT * (mt + 1))
                dst = out[:, srows, :, :].rearrange("b s t d -> (s t) b d")
                nc.sync.dma_start(out=dst, in_=o_t[:])

    nc.compile()
    return nc


def _build_compact():
    """Variant for the (expected) tiled AA_mask: mask[m, k] depends only on
    (m % Ns, k % Nc), so each core loads a tiny per-m-tile [128, Nc] mask
    instead of the full 16.8MB shard -- per-core HBM reads drop ~33%.

    Schedule: a "triangle" of the first 3 m-tiles accumulates both batch
    halves against X tiles as they stream in (6 one-bank PSUM accumulators
    + 2 transpose-staging banks = all of PSUM), so the in-order PE stream
    has matmul work throughout the h/gcw stream. The remaining 5 m-tiles
    run as a PE-bound sequential pipeline fed by trailing gcw loads, which
    have large arrival slack by then."""
    nc = bacc.Bacc(
        "TRN2",
        target_bir_lowering=False,
        debug=False,
        enable_asserts=False,
        num_devices=8,
        num_swdge_queues=2,
    )

    gcw = nc.dram_tensor("gcw", [M_SH, K], F32, kind="ExternalInput").ap()
    msk = nc.dram_tensor("msk", [128, MT * NC_], F32, kind="ExternalInput").ap()
    gcb = nc.dram_tensor("gcb", [M_SH, DH], F32, kind="ExternalInput").ap()
    h = nc.dram_tensor("h", [B_SH, NC_, NT, DH], F32, kind="ExternalInput").ap()
    out = nc.dram_tensor("out", [B_SH, NS, T_SH, DH], F32, kind="ExternalOutput").ap()

    gcw_p = gcw.rearrange("(t s) k -> s t k", t=T_SH)
    gcb_p = gcb.rearrange("(t s) d -> s t d", t=T_SH)

    NTRI = 4  # m-tiles in the streaming triangle (both batch halves)

    with tile.TileContext(nc) as tc:
        with ExitStack() as ctx:
            ident_pool = ctx.enter_context(tc.tile_pool(name="ident", bufs=1))
            x_pool = ctx.enter_context(tc.tile_pool(name="x", bufs=KT))
            gw_pool = ctx.enter_context(tc.tile_pool(name="gw", bufs=4))
            msk_pool = ctx.enter_context(tc.tile_pool(name="msk", bufs=1))
            am_pool = ctx.enter_context(tc.tile_pool(name="am", bufs=2))
            at_pool = ctx.enter_context(tc.tile_pool(name="at", bufs=20))
            gcb_pool = ctx.enter_context(tc.tile_pool(name="gcb", bufs=MT))
            out_pool = ctx.enter_context(tc.tile_pool(name="out", bufs=4))
            ps_pool = ctx.enter_context(
                tc.tile_pool(name="ps", bufs=8, space="PSUM")
            )

            ident = ident_pool.tile([128, 128], BF16)
            masks.make_identity(nc, ident[:])

            gcb_tiles, gw_tiles, x_tiles, at_tiles = [], [], [], {}
            pms = {}

            msk_f32 = msk_pool.tile([128, MT * NC_], F32)
            nc.sync.dma_start(out=msk_f32[:], in_=msk)
            msk_all = msk_pool.tile([128, MT * NC_], BF16)
            nc.vector.tensor_copy(msk_all[:], msk_f32[:])
            msk_tiles = [
                msk_all[:, NC_ * i : NC_ * (i + 1)] for i in range(MT)
            ]

            def emit_gw_dma(mt):
                srows = slice(S_PT * mt, S_PT * (mt + 1))
                gw_t = gw_pool.tile([128, K], BF16, tag="gw", name=f"gw_{mt}")
                nc.gpsimd.dma_start(out=gw_t[:], in_=gcw_p[srows])
                gw_tiles.append(gw_t)

            def emit_x_dmas(r):
                for j in range(4 * r, 4 * r + 4):
                    xt = x_pool.tile([128, NFREE], BF16, tag="x", name=f"x_{j}")
                    src = h[:, 2 * j : 2 * j + 2, :, :].rearrange(
                        "b c t d -> (c t) b d"
                    )
                    nc.gpsimd.dma_start(out=xt[:], in_=src)
                    x_tiles.append(xt)

            def emit_prep(mt):
                am_t = am_pool.tile([128, K], BF16, tag="am", name=f"am_{mt}")
                at_q = [
                    at_pool.tile([128, K // 4], BF16, tag="at", name=f"at_{mt}_{q}")
                    for q in range(4)
                ]
                # am[m, c*Nt+t] = max(gw[m, t*Nc+c], 0) * mask[m, c], in
                # c-quarters so transposes start after 1/4 of the DVE work
                for ch in range(4):
                    cs = slice(NC_ // 4 * ch, NC_ // 4 * (ch + 1))
                    ks = slice(K // 4 * ch, K // 4 * (ch + 1))
                    nc.vector.scalar_tensor_tensor(
                        am_t[:, ks].rearrange("m (c t) -> m t c", c=NC_ // 4),
                        gw_tiles[mt][:].rearrange("m (t c) -> m t c", c=NC_)[
                            :, :, cs
                        ],
                        0.0,
                        msk_tiles[mt][:, cs].unsqueeze(1).broadcast_to(
                            (128, NT, NC_ // 4)
                        ),
                        mybir.AluOpType.max,
                        mybir.AluOpType.mult,
                    )
                    for g in range(ch, ch + 1):
                        ptr = ps_pool.tile(
                            [128, 1024], BF16, tag="ps", name=f"ptr_{g}"
                        )
                        for j8 in range(8):
                            j = 8 * g + j8
                            nc.tensor.transpose(
                                ptr[:, 128 * j8 : 128 * j8 + 128],
                                am_t[:, 128 * j : 128 * j + 128],
                                ident[:],
                            )
                        dstslice = at_q[g][:]
                        if g % 2 == 0:
                            nc.scalar.copy(dstslice, ptr[:])
                        else:
                            nc.vector.tensor_copy(dstslice, ptr[:])
                at_tiles[mt] = at_q

            def emit_mms(mt, ks, bh):
                pm = pms[(mt, bh)]
                at_q = at_tiles[mt]
                for k in ks:
                    q, kq = k // 8, k % 8
                    nc.tensor.matmul(
                        pm[:],
                        at_q[q][:, 128 * kq : 128 * kq + 128],
                        x_tiles[k][:, 512 * bh : 512 * bh + 512],
                        start=(k == 0),
                        stop=(k == KT - 1),
                    )

            def emit_epi(mt, bh):
                pm = pms.pop((mt, bh))
                o_t = out_pool.tile([128, 512], F32, tag="out", name=f"o_{mt}_{bh}")
                bias = gcb_tiles[mt][:].unsqueeze(1).broadcast_to(
                    (128, 4, DH)
                )
                nc.vector.tensor_add(
                    o_t[:].rearrange("p (b d) -> p b d", b=4),
                    pm[:].rearrange("p (b d) -> p b d", b=4),
                    bias,
                )
                nc.scalar.activation(
                    o_t[:], o_t[:], mybir.ActivationFunctionType.Relu
                )
                srows = slice(S_PT * mt, S_PT * (mt + 1))
                dst = out[4 * bh : 4 * bh + 4, srows, :, :].rearrange(
                    "b s t d -> (s t) b d"
                )
                nc.sync.dma_start(out=dst, in_=o_t[:])

            def alloc_pm(mt, bh):
                pms[(mt, bh)] = ps_pool.tile(
                    [128, 512], F32, tag="ps", name=f"pm_{mt}_{bh}"
                )

            # ---- DMA + compute emission ----
            # streaming phase: gcw(0..2) early, X windows, triangle MMs
            for r in range(MT):
                if r < NTRI:
                    emit_gw_dma(r)
                if r >= 6 and NTRI + (r - 6) < MT:
                    emit_gw_dma(NTRI + (r - 6))  # early trailing gcw
                emit_x_dmas(r)
                if r == 2:
                    for i in range(MT):
                        srows2 = slice(S_PT * i, S_PT * (i + 1))
                        gcb_t = gcb_pool.tile(
                            [128, DH], F32, tag="gcb", name=f"gcb_{i}"
                        )
                        nc.sync.dma_start(out=gcb_t[:], in_=gcb_p[srows2])
                        gcb_tiles.append(gcb_t)
                if r < NTRI:
                    if r < NTRI - 1:
                        # allocate ahead of the prep's ptr tiles so the
                        # accumulators land on distinct PSUM slots (avoids a
                        # slot WAR stalling the first catch-up matmuls)
                        alloc_pm(r, 0)
                        alloc_pm(r, 1)
                    emit_prep(r)
                for mt in range(min(r, NTRI - 1) + 1):
                    if mt == r:
                        if (mt, 0) not in pms:
                            alloc_pm(mt, 0)
                            alloc_pm(mt, 1)
                        ks = range(0, 4 * r + 4)
                    else:
                        ks = range(4 * r, 4 * r + 4)
                    for k in ks:
                        for bh in range(2):
                            emit_mms(mt, [k], bh)

            # remaining trailing gcw loads: needed only as the sequential
            # tail consumes them, well after the X stream completes
            for mt in range(NTRI + 2, MT):
                emit_gw_dma(mt)

            # triangle epilogues, then the PE-bound sequential tail
            for mt in range(NTRI):
                emit_epi(mt, 0)
                emit_epi(mt, 1)
            for mt in range(NTRI, MT):
                emit_prep(mt)
                for bh in range(2):
                    alloc_pm(mt, bh)
                    emit_mms(mt, range(KT), bh)
                    emit_epi(mt, bh)

    nc.compile()
    return nc


def _build_full_tri():
    """General-mask fallback with the same triangular schedule: streams
    the full AA shard alongside GCW (both bf16-cast in the DMA)."""
    nc = bacc.Bacc(
        "TRN2",
        target_bir_lowering=False,
        debug=False,
        enable_asserts=False,
        num_devices=8,
        num_swdge_queues=2,
    )

    gcw = nc.dram_tensor("gcw", [M_SH, K], F32, kind="ExternalInput").ap()
    aa = nc.dram_tensor("aa", [M_SH, K], F32, kind="ExternalInput").ap()
    gcb = nc.dram_tensor("gcb", [M_SH, DH], F32, kind="ExternalInput").ap()
    h = nc.dram_tensor("h", [B_SH, NC_, NT, DH], F32, kind="ExternalInput").ap()
    out = nc.dram_tensor("out", [B_SH, NS, T_SH, DH], F32, kind="ExternalOutput").ap()

    gcw_p = gcw.rearrange("(t s) k -> s t k", t=T_SH)
    aa_p = aa.rearrange("(t s) k -> s t k", t=T_SH)
    gcb_p = gcb.rearrange("(t s) d -> s t d", t=T_SH)

    NTRI = 4  # m-tiles in the streaming triangle (both batch halves)

    with tile.TileContext(nc) as tc:
        with ExitStack() as ctx:
            ident_pool = ctx.enter_context(tc.tile_pool(name="ident", bufs=1))
            x_pool = ctx.enter_context(tc.tile_pool(name="x", bufs=KT))
            gw_pool = ctx.enter_context(tc.tile_pool(name="gw", bufs=4))
            aa_pool = ctx.enter_context(tc.tile_pool(name="aam", bufs=4))
            am_pool = ctx.enter_context(tc.tile_pool(name="am", bufs=2))
            at_pool = ctx.enter_context(tc.tile_pool(name="at", bufs=20))
            gcb_pool = ctx.enter_context(tc.tile_pool(name="gcb", bufs=MT))
            out_pool = ctx.enter_context(tc.tile_pool(name="out", bufs=4))
            ps_pool = ctx.enter_context(
                tc.tile_pool(name="ps", bufs=8, space="PSUM")
            )

            ident = ident_pool.tile([128, 128], BF16)
            masks.make_identity(nc, ident[:])

            gcb_tiles, gw_tiles, x_tiles, at_tiles = [], [], [], {}
            pms = {}

            aa_tiles = []

            def emit_gw_dma(mt):
                srows = slice(S_PT * mt, S_PT * (mt + 1))
                gw_t = gw_pool.tile([128, K], BF16, tag="gw", name=f"gw_{mt}")
                nc.gpsimd.dma_start(out=gw_t[:], in_=gcw_p[srows])
                gw_tiles.append(gw_t)
                aa_t = aa_pool.tile([128, K], BF16, tag="aa", name=f"aa_{mt}")
                nc.gpsimd.dma_start(out=aa_t[:], in_=aa_p[srows])
                aa_tiles.append(aa_t)

            def emit_x_dmas(r):
                for j in range(4 * r, 4 * r + 4):
                    xt = x_pool.tile([128, NFREE], BF16, tag="x", name=f"x_{j}")
                    src = h[:, 2 * j : 2 * j + 2, :, :].rearrange(
                        "b c t d -> (c t) b d"
                    )
                    nc.gpsimd.dma_start(out=xt[:], in_=src)
                    x_tiles.append(xt)

            def emit_prep(mt):
                am_t = am_pool.tile([128, K], BF16, tag="am", name=f"am_{mt}")
                at_q = [
                    at_pool.tile([128, K // 4], BF16, tag="at", name=f"at_{mt}_{q}")
                    for q in range(4)
                ]
                # am[m, c*Nt+t] = max(gw[m, t*Nc+c], 0) * mask[m, c], in
                # c-quarters so transposes start after 1/4 of the DVE work
                for ch in range(4):
                    cs = slice(NC_ // 4 * ch, NC_ // 4 * (ch + 1))
                    ks = slice(K // 4 * ch, K // 4 * (ch + 1))
                    nc.vector.scalar_tensor_tensor(
                        am_t[:, ks].rearrange("m (c t) -> m t c", c=NC_ // 4),
                        gw_tiles[mt][:].rearrange("m (t c) -> m t c", c=NC_)[
                            :, :, cs
                        ],
                        0.0,
                        aa_tiles[mt][:].rearrange(
                            "m (t c) -> m t c", c=NC_
                        )[:, :, cs],
                        mybir.AluOpType.max,
                        mybir.AluOpType.mult,
                    )
                    for g in range(ch, ch + 1):
                        ptr = ps_pool.tile(
                            [128, 1024], BF16, tag="ps", name=f"ptr_{g}"
                        )
                        for j8 in range(8):
                            j = 8 * g + j8
                            nc.tensor.transpose(
                                ptr[:, 128 * j8 : 128 * j8 + 128],
                                am_t[:, 128 * j : 128 * j + 128],
                                ident[:],
                            )
                        dstslice = at_q[g][:]
                        if g % 2 == 0:
                            nc.scalar.copy(dstslice, ptr[:])
                        else:
                            nc.vector.tensor_copy(dstslice, ptr[:])
                at_tiles[mt] = at_q

            def emit_mms(mt, ks, bh):
                pm = pms[(mt, bh)]
                at_q = at_tiles[mt]
                for k in ks:
                    q, kq = k // 8, k % 8
                    nc.tensor.matmul(
                        pm[:],
                        at_q[q][:, 128 * kq : 128 * kq + 128],
                        x_tiles[k][:, 512 * bh : 512 * bh + 512],
                        start=(k == 0),
                        stop=(k == KT - 1),
                    )

            def emit_epi(mt, bh):
                pm = pms.pop((mt, bh))
                o_t = out_pool.tile([128, 512], F32, tag="out", name=f"o_{mt}_{bh}")
                bias = gcb_tiles[mt][:].unsqueeze(1).broadcast_to(
                    (128, 4, DH)
                )
                nc.vector.tensor_add(
                    o_t[:].rearrange("p (b d) -> p b d", b=4),
                    pm[:].rearrange("p (b d) -> p b d", b=4),
                    bias,
                )
                nc.scalar.activation(
                    o_t[:], o_t[:], mybir.ActivationFunctionType.Relu
                )
                srows = slice(S_PT * mt, S_PT * (mt + 1))
                dst = out[4 * bh : 4 * bh + 4, srows, :, :].rearrange(
                    "b s t d -> (s t) b d"
                )
                nc.sync.dma_start(out=dst, in_=o_t[:])

            def alloc_pm(mt, bh):
                pms[(mt, bh)] = ps_pool.tile(
                    [128, 512], F32, tag="ps", name=f"pm_{mt}_{bh}"
                )

            # ---- DMA + compute emission ----
            # streaming phase: gcw(0..2) early, X windows, triangle MMs
            for r in range(MT):
                if r < NTRI:
                    emit_gw_dma(r)
                if r >= 6 and NTRI + (r - 6) < MT:
                    emit_gw_dma(NTRI + (r - 6))  # early trailing gcw
                emit_x_dmas(r)
                if r == 2:
                    for i in range(MT):
                        srows2 = slice(S_PT * i, S_PT * (i + 1))
                        gcb_t = gcb_pool.tile(
                            [128, DH], F32, tag="gcb", name=f"gcb_{i}"
                        )
                        nc.sync.dma_start(out=gcb_t[:], in_=gcb_p[srows2])
                        gcb_tiles.append(gcb_t)
                if r < NTRI:
                    if r < NTRI - 1:
                        # allocate ahead of the prep's ptr tiles so the
                        # accumulators land on distinct PSUM slots (avoids a
                        # slot WAR stalling the first catch-up matmuls)
                        alloc_pm(r, 0)
                        alloc_pm(r, 1)
                    emit_prep(r)
                for mt in range(min(r, NTRI - 1) + 1):
                    if mt == r:
                        if (mt, 0) not in pms:
                            alloc_pm(mt, 0)
                            alloc_pm(mt, 1)
                        ks = range(0, 4 * r + 4)
                    else:
                        ks = range(4 * r, 4 * r + 4)
                    for k in ks:
                        for bh in range(2):
                            emit_mms(mt, [k], bh)

            # remaining trailing gcw loads: needed only as the sequential
            # tail consumes them, well after the X stream completes
            for mt in range(NTRI + 2, MT):
                emit_gw_dma(mt)

            # triangle epilogues, then the PE-bound sequential tail
            for mt in range(NTRI):
                emit_epi(mt, 0)
                emit_epi(mt, 1)
            for mt in range(NTRI, MT):
                emit_prep(mt)
                for bh in range(2):
                    alloc_pm(mt, bh)
                    emit_mms(mt, range(KT), bh)
                    emit_epi(mt, bh)

    nc.compile()
    return nc




def _mask_small(AA_mask):
    """[128, MT*Nc] per-m'-tile mask rows, mt-major along the free dim
    (identical for every core)."""
    A64 = AA_mask[:NS, :NC_]
    ms = np.empty((128, MT * NC_), dtype=np.float32)
    for mt in range(MT):
        for p in range(128):
            s = S_PT * mt + p // T_SH
            ms[p, NC_ * mt : NC_ * (mt + 1)] = A64[s]
    return ms


def _is_tiled(AA_mask):
    A64 = AA_mask[:NS, :NC_]
    return np.array_equal(AA_mask, np.tile(A64, (NT, NT)))


def _make_in_maps(h, AA_mask, GCW, GCB, compact):
    in_maps = []
    ms = _mask_small(AA_mask) if compact else None
    for r in range(8):
        rq, bq = r % P_ROW, r // P_ROW
        rs = slice(M_SH * rq, M_SH * (rq + 1))
        bs_ = slice(B_SH * bq, B_SH * (bq + 1))
        m = {
            "gcw": np.ascontiguousarray(GCW[rs], np.float32),
            "gcb": np.ascontiguousarray(GCB[rs], np.float32),
            "h": np.ascontiguousarray(h[bs_], np.float32),
        }
        if compact:
            m["msk"] = ms
        else:
            m["aa"] = np.ascontiguousarray(AA_mask[rs], np.float32)
        in_maps.append(m)
    return in_maps


def _assemble(results):
    full = np.empty((BS, NS, NT, DH), dtype=np.float32)
    for r in range(8):
        rq, bq = r % P_ROW, r // P_ROW
        full[
            B_SH * bq : B_SH * (bq + 1), :, T_SH * rq : T_SH * (rq + 1), :
        ] = results[r]["out"]
    return full


def kernel(h, e, AA_mask, GCW, GCB):
    h = np.asarray(h)
    AA_mask = np.asarray(AA_mask)
    GCW = np.asarray(GCW)
    GCB = np.asarray(GCB)

    compact = _is_tiled(AA_mask)
    key = "compact" if compact else "full"
    if key not in _cached:
        if compact:
            _cached[key] = _build_compact()
        else:
            try:
                _cached[key] = _build_full_tri()
            except Exception:
                _cached[key] = _build()
    nc = _cached[key]

    in_maps = _make_in_maps(h, AA_mask, GCW, GCB, compact)
    res = run_bass_kernel_spmd(nc, in_maps, core_ids=list(range(8)))
    return _assemble(res.results)



# revision 2
# speedup vs baseline: 1.1732x; 1.1732x over previous
"""GCNFast Trainium2 kernel, v2 (tiled-mask fast path).

out[b] = relu(A @ x_b + GCB),  A = relu(AA_mask * GCW)  [4096, 4096]

Sharding: 4-way t-shard of output rows x 2-way batch shard (8 cores).
Row order per core: m' = s_rank*16 + t_local, so every core's m-tile i
covers the SAME 8 s-values -> mask-zero (s-group, c-pair) blocks skip
identically on all cores (SPMD-safe block sparsity, ~21% of matmuls).

Per core:
 - GCW m-tile slices are host-gathered (rows in m'-order, columns c-major
   restricted to the chunks that m-tile actually needs) and DMA-TRANSPOSED
   from DRAM straight into A^T layout via the xbar path (no PE transposes,
   no separate weight load).
 - DVE applies relu*mask in transposed space (one fused STT per piece).
 - PE runs only the needed 128x128x512 bf16 matmuls, accumulating in PSUM.
 - Epilogue: DVE bias-add (PSUM+GCB -> bf16), ACT relu, ACT hwdge store.
 - PE warm-up transposes cover the initial DMA latency so the ramp model
   reaches full clock before the real matmul stream starts.
"""

from contextlib import ExitStack

import numpy as np
import ml_dtypes

import concourse.mybir as mybir
import concourse.tile as tile
from concourse import bacc, masks
from concourse.bass_utils import run_bass_kernel_spmd

NC_, NS, NT, DH, BS = 64, 64, 64, 128, 16
K = NC_ * NT
M = NS * NT
P_ROW, P_BATCH = 4, 2
M_SH = M // P_ROW      # 1024 rows per core
B_SH = BS // P_BATCH   # 8 batches per core
T_SH = NT // P_ROW     # 16 t per core
NFREE = B_SH * DH      # 1024
N_MT = 8               # m-tiles per core
S_PT = 8               # s-values per m-tile

F32 = mybir.dt.float32
BF16 = mybir.dt.bfloat16
NBF = ml_dtypes.bfloat16

_cached = {}
_meta_cache = {}

N_B = 3  # m-tiles computed during the X streaming phase


def _optimize_perms(U, iters=40000, restarts=4):
    """Plateau-walking local search over s-permutation (8-groups) and
    c-permutation (adjacent pairs), minimizing needed (m-tile, chunk)
    matmul blocks. Deterministic; best of several restarts."""

    def cost(sp, cp):
        Us = U[sp].reshape(8, 8, 64).any(1)       # [mt, c]
        Uc = Us[:, cp].reshape(8, 32, 2).any(2)   # [mt, chunk]
        return int(Uc.sum()), Uc

    bb = None
    for seed in range(restarts):
        rng = np.random.default_rng(seed)
        s_perm = (np.arange(64) if seed == 0
                  else rng.permutation(64))
        c_perm = np.arange(64)
        best, used = cost(s_perm, c_perm)
        for _ in range(iters):
            if rng.random() < 0.5:
                p = s_perm.copy()
                i, j = rng.integers(0, 64, 2)
                p[i], p[j] = p[j], p[i]
                c, u = cost(p, c_perm)
                if c <= best:
                    best, s_perm, used = c, p, u
            else:
                p = c_perm.copy()
                i, j = rng.integers(0, 64, 2)
                p[i], p[j] = p[j], p[i]
                c, u = cost(s_perm, p)
                if c <= best:
                    best, c_perm, used = c, p, u
        if bb is None or best < bb[0]:
            bb = (best, s_perm, c_perm, used)
    return bb[1], bb[2], bb[3]


PIECE_CAP = 12  # max chunks per gwT transpose / STT piece


def _split_sizes(n, first):
    """Piece sizes for one m-tile's gwT/STT splitting (each <= PIECE_CAP)."""
    out = []
    rem = n
    for f in first:
        if rem <= 0:
            break
        s = min(f, rem, PIECE_CAP)
        out.append(s)
        rem -= s
    while rem > 0:
        s = min(PIECE_CAP, rem)
        out.append(s)
        rem -= s
    return out


def _x_group_sizes(n):
    sizes = []
    pat = _knob("KV2_XPAT", [1, 1, 1, 2, 3])
    i = 0
    while sum(sizes) < n:
        sizes.append(pat[i] if i < len(pat) else 3)
        i += 1
    sizes[-1] -= sum(sizes) - n
    if sizes[-1] == 0:
        sizes.pop()
    return sizes


import os


def _knob(name, default):
    v = os.environ.get(name)
    return default if v is None else eval(v)


def _make_meta(A64):
    U = (A64 != 0)
    key = U.tobytes()
    if key in _meta_cache:
        return _meta_cache[key]
    s_perm, c_perm, used = _optimize_perms(U)
    # order the 8 s-groups so the X-phase (first N_B slots) has the most
    # PE work per streamed chunk
    sizes = used.sum(1)
    order = np.argsort(-sizes, kind="stable")
    s_perm = s_perm.reshape(8, 8)[order].reshape(64)
    used = used[order]
    L = [list(np.flatnonzero(used[mt])) for mt in range(N_MT)]
    kx = sorted(set().union(*[set(l) for l in L])) if any(L) else []
    xslot = {j: i for i, j in enumerate(kx)}
    nb = _knob("KV2_NB", N_B)
    pieces = []
    for mt in range(N_MT):
        if mt == 0:
            pieces.append(_split_sizes(len(L[mt]), _knob("KV2_P0", [4, 8])))
        elif mt < nb:
            pieces.append(_split_sizes(len(L[mt]), _knob("KV2_PB", [8])))
        else:
            pieces.append(_split_sizes(len(L[mt]), []))
    meta = {
        "s_perm": s_perm,
        "c_perm": c_perm,
        "L": L,
        "kx": kx,
        "xslot": xslot,
        "pieces": pieces,
        "xgroups": _x_group_sizes(len(kx)),
        "n_warm": _knob("KV2_NWARM", 30),
        "n_b": nb,
        "gwt_bufs": _knob("KV2_GWTBUFS", 5),
    }
    _meta_cache[key] = meta
    return meta


def _build_key(meta):
    return (
        "v2",
        tuple(tuple(l) for l in meta["L"]),
        tuple(meta["kx"]),
        tuple(tuple(p) for p in meta["pieces"]),
        tuple(meta["xgroups"]),
        meta["n_warm"],
    )


def _build_v2(meta):
    L = meta["L"]
    kx = meta["kx"]
    xslot = meta["xslot"]
    pieces = meta["pieces"]
    xgroups = meta["xgroups"]
    nkx = len(kx)
    n_b = meta.get("n_b", N_B)

    nc = bacc.Bacc(
        "TRN2",
        target_bir_lowering=False,
        debug=False,
        enable_asserts=False,
        num_devices=8,
        num_swdge_queues=2,
    )

    gwt_d = [
        nc.dram_tensor(f"gwt{mt}", [128, 128 * max(len(L[mt]), 1)], BF16,
                       kind="ExternalInput").ap()
        for mt in range(N_MT)
    ]
    msk_d = [
        nc.dram_tensor(f"msk{mt}", [128, max(len(L[mt]), 1) * S_PT], BF16,
                       kind="ExternalInput").ap()
        for mt in range(N_MT)
    ]
    hx = nc.dram_tensor("hx", [2 * max(nkx, 1), NT, B_SH, DH], F32,
                        kind="ExternalInput").ap()
    gcb = nc.dram_tensor("gcb", [M_SH, DH], F32, kind="ExternalInput").ap()
    out = nc.dram_tensor("out", [2, M_SH, 4 * DH], BF16,
                         kind="ExternalOutput").ap()

    n_pieces = sum(len(p) for p in pieces)

    max_l = max(max((len(l) for l in L), default=1), 1)
    max_piece = max(
        max((max(p) for p in pieces if p), default=1), 1
    )

    with tile.TileContext(nc) as tc:
        with ExitStack() as ctx:
            junk_pool = ctx.enter_context(tc.tile_pool(name="junk", bufs=1))
            x_pool = ctx.enter_context(
                tc.tile_pool(name="x", bufs=max(len(xgroups), 1))
            )
            gwt_pool = ctx.enter_context(tc.tile_pool(name="gwt", bufs=meta.get("gwt_bufs", 5)))
            at_pool = ctx.enter_context(tc.tile_pool(name="at", bufs=N_MT))
            msk_pool = ctx.enter_context(tc.tile_pool(name="msk", bufs=N_MT))
            gcb_pool = ctx.enter_context(tc.tile_pool(name="gcb", bufs=1))
            o_pool = ctx.enter_context(tc.tile_pool(name="o", bufs=4))
            pacer_pool = ctx.enter_context(tc.tile_pool(name="pcr", bufs=1))
            ps_pool = ctx.enter_context(
                tc.tile_pool(name="ps", bufs=8, space="PSUM")
            )

            # junk operand for PE warm-up matmuls (content irrelevant)
            junk = junk_pool.tile([128, 128], BF16)
            nc.gpsimd.memset(junk[:], 0)

            msk_tiles = [None] * N_MT
            at_tiles = [None] * N_MT   # per mt: [128, L, 128]
            gwt_info = {}              # (mt, pi) -> (gwt_tile, off, ln)
            pms = {}
            o_tiles = {}
            pacer = pacer_pool.tile([128, 64], BF16)

            def emit_msk(mt):
                lm = len(L[mt])
                if lm == 0:
                    return
                mk = msk_pool.tile([128, max_l, S_PT], BF16, name=f"mk{mt}")
                nc.sync.dma_start(
                    out=mk[:, :lm, :],
                    in_=msk_d[mt].rearrange("p (l s) -> p l s", s=S_PT),
                )
                msk_tiles[mt] = mk
                at = at_pool.tile([128, max_l, 128], BF16,
                                  tag="at", name=f"at{mt}")
                at_tiles[mt] = at

            def emit_gwt_piece(mt, pi):
                ln = pieces[mt][pi]
                off = sum(pieces[mt][:pi])
                gt = gwt_pool.tile([128, max_piece, 128], BF16,
                                   tag="gwt", name=f"gw{mt}_{pi}")
                nc.sync.dma_start(
                    out=gt[:, :ln, :],
                    in_=gwt_d[mt][:, 128 * off: 128 * (off + ln)]
                    .rearrange("p (l m) -> p l m", m=128),
                )
                gwt_info[(mt, pi)] = (gt, off, ln)

            def emit_stt_piece(mt, pi):
                mk = msk_tiles[mt]
                at = at_tiles[mt]
                gt, off, ln = gwt_info[(mt, pi)]
                nc.vector.scalar_tensor_tensor(
                    at[:, off: off + ln, :]
                    .rearrange("p l (s t) -> p l s t", s=S_PT),
                    gt[:, :ln, :]
                    .rearrange("p l (s t) -> p l s t", s=S_PT),
                    0.0,
                    mk[:, off: off + ln, :]
                    .unsqueeze(3)
                    .broadcast_to((128, ln, S_PT, T_SH)),
                    mybir.AluOpType.max,
                    mybir.AluOpType.mult,
                )

            def emit_gwt(mt):
                emit_msk(mt)
                for pi in range(len(pieces[mt])):
                    emit_gwt_piece(mt, pi)

            def emit_stt(mt):
                if len(L[mt]) == 0:
                    return
                for pi in range(len(pieces[mt])):
                    emit_stt_piece(mt, pi)

            x_tiles = []
            slot_tile = {}

            def emit_x():
                s0 = 0
                for gi, g in enumerate(xgroups):
                    xt = x_pool.tile([128, max(xgroups), NFREE], BF16,
                                     tag="x", name=f"x{gi}")
                    cs = 2 * s0
                    src = hx[cs: cs + 2 * g, :, :, :].rearrange(
                        "(g c) t b d -> (c t) g b d", c=2
                    )
                    nc.gpsimd.dma_start(out=xt[:, :g, :], in_=src)
                    for i in range(g):
                        slot_tile[s0 + i] = (xt, i)
                    x_tiles.append(xt)
                    s0 += g

            def emit_pacer(gi):
                """Tiny SBUF->SBUF DMA on sync that waits for X group gi:
                paces later sync DMA issues behind the X stream."""
                nc.sync.dma_start(out=pacer[:], in_=x_tiles[gi][:, 0, 0:64])

            def emit_mm_h(mt, u, hh, first, last):
                at = at_tiles[mt]
                j = L[mt][u]
                xt, i = slot_tile[xslot[j]]
                nc.tensor.matmul(
                    pms[(mt, hh)][:],
                    at[:, u, :],
                    xt[:, i, 512 * hh: 512 * (hh + 1)],
                    start=first,
                    stop=last,
                )

            def emit_mm(mt, u, first, last):
                for hh in range(2):
                    emit_mm_h(mt, u, hh, first, last)

            def emit_bias(mt, hh, half=None):
                """DVE: o_t = psum + bias (bf16). half=(0|1) does 256 cols."""
                key = (mt, hh)
                if half in (None, 0):
                    o_t = o_pool.tile([128, 512], BF16, tag="o",
                                      name=f"o{mt}_{hh}")
                    o_tiles[key] = o_t
                o_t = o_tiles[key]
                if half is None:
                    cs, nb = slice(0, 512), 4
                else:
                    cs, nb = slice(256 * half, 256 * (half + 1)), 2
                bias = (
                    gcb_t[:, mt, :].unsqueeze(1).broadcast_to((128, nb, DH))
                )
                if key in pms:
                    pm = pms[key] if half == 0 else (
                        pms.pop(key) if half in (None, 1) else pms[key]
                    )
                    nc.vector.tensor_add(
                        o_t[:, cs].rearrange("p (b d) -> p b d", b=nb),
                        pm[:, cs].rearrange("p (b d) -> p b d", b=nb),
                        bias,
                    )
                else:  # fully-masked m-tile: out = relu(bias)
                    nc.vector.tensor_copy(
                        o_t[:, cs].rearrange("p (b d) -> p b d", b=nb), bias
                    )

            def emit_relu_store(mt, hh, half=None):
                key = (mt, hh)
                o_t = o_tiles[key] if half == 0 else o_tiles.pop(key)
                if half is None:
                    cs = slice(0, 512)
                    dst = out[hh, 128 * mt: 128 * (mt + 1), :]
                else:
                    cs = slice(256 * half, 256 * (half + 1))
                    dst = out[hh, 128 * mt: 128 * (mt + 1), cs]
                nc.scalar.activation(
                    o_t[:, cs], o_t[:, cs], mybir.ActivationFunctionType.Relu
                )
                nc.scalar.dma_start(out=dst, in_=o_t[:, cs])

            def alloc_pm(mt):
                for hh in range(2):
                    pms[(mt, hh)] = ps_pool.tile(
                        [128, 512], F32, tag="ps", name=f"pm{mt}_{hh}"
                    )

            # ---- emission ----
            # B-phase masks + first gwt pieces, round-robin across m-tiles
            for mt in range(n_b):
                emit_msk(mt)
                emit_gwt_piece(mt, 0)
            emit_x()
            # PE warm-up: junk matmuls into pm(0,0) keep the tensor engine
            # ramping while the first weights/X stream in. start/stop=True
            # each, so the real accumulation (start=True) overwrites.
            if L[0]:
                alloc_pm(0)
                for _ in range(meta["n_warm"]):
                    nc.tensor.matmul(
                        pms[(0, 0)][:, :128], junk[:], junk[:],
                        start=True, stop=True, skip_group_check=True,
                    )
            for mt in range(n_b):
                emit_stt_piece(mt, 0)

            # remaining B pieces in need order (earliest-needed first)
            blist = []
            for mt in range(n_b):
                if not L[mt]:
                    continue
                for pi in range(1, len(pieces[mt])):
                    off = sum(pieces[mt][:pi])
                    need = int(off * len(kx) / max(len(L[mt]), 1))
                    blist.append((need, mt, pi))
            blist.sort()
            for need, mt, pi in blist:
                emit_gwt_piece(mt, pi)
                emit_stt_piece(mt, pi)

            # phase B: chunk-major over the first n_b m-tiles as X streams in.
            # Junk matmuls after each of the first JW chunks pad X-delivery
            # jitter so the PE (and its ramp clock) never goes idle.
            for mt in range(1, n_b):
                if L[mt]:
                    alloc_pm(mt)
            jw = _knob("KV2_JW", 0)
            jn = _knob("KV2_JN", 2)
            scratch = ps_pool.tile([128, 512], F32, tag="ps", name="scratch")
            nxt = [0] * n_b   # next u per B m-tile
            for ji, j in enumerate(kx):
                for mt in range(n_b):
                    if nxt[mt] < len(L[mt]) and L[mt][nxt[mt]] == j:
                        u = nxt[mt]
                        emit_mm(mt, u, u == 0, u == len(L[mt]) - 1)
                        nxt[mt] += 1
                if ji < jw:
                    for _ in range(jn):
                        nc.tensor.matmul(
                            scratch[:, :128], junk[:], junk[:],
                            start=True, stop=True, skip_group_check=True,
                        )

            # C-phase weights: pacer keeps them behind the X stream on the
            # shared DMA device
            emit_pacer(len(xgroups) - 1)
            gcb_t = gcb_pool.tile([128, N_MT, DH], F32)
            nc.sync.dma_start(
                out=gcb_t[:], in_=gcb.rearrange("(m p) d -> p m d", p=128)
            )
            for mt in range(n_b, N_MT):
                emit_gwt(mt)
            for mt in range(n_b, min(n_b + 2, N_MT)):
                emit_stt(mt)

            # B epilogues, interleaved with remaining STTs
            stt_next = n_b + 2
            for mt in range(n_b):
                for hh in range(2):
                    emit_bias(mt, hh)
                for hh in range(2):
                    emit_relu_store(mt, hh)
                if stt_next < N_MT:
                    emit_stt(stt_next)
                    stt_next += 1
            while stt_next < N_MT:
                emit_stt(stt_next)
                stt_next += 1

            # phase C: m-tile-major, h-major so h0's epilogue overlaps h1
            for mt in range(n_b, N_MT):
                if L[mt]:
                    alloc_pm(mt)
                    for hh in range(2):
                        last = mt == N_MT - 1 and hh == 1
                        for u in range(len(L[mt])):
                            emit_mm_h(mt, u, hh, u == 0, u == len(L[mt]) - 1)
                        if last:  # split the final epilogue for a short tail
                            emit_bias(mt, hh, 0)
                            emit_relu_store(mt, hh, 0)
                            emit_bias(mt, hh, 1)
                            emit_relu_store(mt, hh, 1)
                        else:
                            emit_bias(mt, hh)
                            emit_relu_store(mt, hh)
                else:
                    for hh in range(2):
                        emit_bias(mt, hh)
                        emit_relu_store(mt, hh)

    nc.compile()
    return nc


def _host_prep(h, A64, GCW, GCB, meta):
    """Shard + reorder inputs for the v2 kernel. Pure data movement
    (slicing, permutation, dtype cast); all model math stays on device."""
    s_perm = meta["s_perm"]
    c_perm = meta["c_perm"]
    L = meta["L"]
    kx = meta["kx"]
    U = (A64 != 0)

    GCWb = GCW.astype(NBF)

    # columns of GCW for chunk j, in k' (c-major) order: 64 t' per c
    tgrid = np.arange(NT)

    def chunk_cols(j):
        c0, c1 = c_perm[2 * j], c_perm[2 * j + 1]
        return np.concatenate([tgrid * NC_ + c0, tgrid * NC_ + c1])

    cols_of = {j: chunk_cols(j) for j in range(32)}

    # global row index for core-row m' of row-shard rq:
    # t = T_SH*rq + (m' % 16), s = s_perm[m' // 16], row = t*64 + s
    mp = np.arange(M_SH)
    rows_rq = [
        (T_SH * rq + (mp % T_SH)) * NS + s_perm[mp // T_SH]
        for rq in range(P_ROW)
    ]

    gwt_arrs = [[None] * N_MT for _ in range(P_ROW)]
    for rq in range(P_ROW):
        rows = rows_rq[rq]
        for mt in range(N_MT):
            r = rows[128 * mt: 128 * (mt + 1)]
            if L[mt]:
                cols = np.concatenate([cols_of[j] for j in L[mt]])
                arr = GCWb[np.ix_(r, cols)]  # [128 m, 128*L k']
                # transposed (lhsT) layout: [128 k'-in-chunk, L, 128 m]
                arr = np.ascontiguousarray(
                    arr.reshape(128, len(L[mt]), 128).transpose(2, 1, 0)
                ).reshape(128, -1)
            else:
                arr = np.zeros((128, 128), dtype=NBF)
            gwt_arrs[rq][mt] = arr

    msk_arrs = []
    for mt in range(N_MT):
        lm = max(len(L[mt]), 1)
        mk = np.zeros((128, lm, S_PT), dtype=np.float32)
        p = np.arange(128)
        for u, j in enumerate(L[mt]):
            cs = np.where(p < NT, c_perm[2 * j], c_perm[2 * j + 1])
            for sx in range(S_PT):
                mk[:, u, sx] = U[s_perm[S_PT * mt + sx], cs]
        msk_arrs.append(
            np.ascontiguousarray(mk.reshape(128, lm * S_PT).astype(NBF))
        )

    c_sel = np.concatenate(
        [[c_perm[2 * j], c_perm[2 * j + 1]] for j in kx]
    ) if kx else np.array([0, 1])
    # [2|kx|, 64 t, 8 b, 128 d] per batch group (b innermost so the DMA
    # access pattern collapses to 3 dims with 4KB contiguous runs)
    hx_groups = [
        np.ascontiguousarray(
            h[B_SH * bq: B_SH * (bq + 1), c_sel, :, :].transpose(1, 2, 0, 3),
            dtype=np.float32,
        )
        for bq in range(P_BATCH)
    ]

    gcb_arrs = [
        np.ascontiguousarray(GCB[rows_rq[rq]], np.float32)
        for rq in range(P_ROW)
    ]

    in_maps = []
    for r in range(8):
        rq, bq = r % P_ROW, r // P_ROW
        m = {
            "hx": hx_groups[bq],
            "gcb": gcb_arrs[rq],
        }
        for mt in range(N_MT):
            m[f"gwt{mt}"] = gwt_arrs[rq][mt]
            m[f"msk{mt}"] = msk_arrs[mt]
        in_maps.append(m)
    return in_maps


def _assemble_v2_fixed(results, meta):
    s_perm = meta["s_perm"]
    inv = np.argsort(s_perm)
    full = np.empty((BS, NS, NT, DH), dtype=np.float32)
    for r in range(8):
        rq, bq = r % P_ROW, r // P_ROW
        o = np.asarray(results[r]["out"]).astype(np.float32)
        o = o.reshape(2, NS, T_SH, 4, DH).transpose(0, 3, 1, 2, 4)
        o = o.reshape(B_SH, NS, T_SH, DH)
        full[B_SH * bq: B_SH * (bq + 1), :,
             T_SH * rq: T_SH * (rq + 1), :] = o[:, inv, :, :]
    return full


def _build():
    nc = bacc.Bacc(
        "TRN2",
        target_bir_lowering=False,
        debug=False,
        enable_asserts=False,
        num_devices=8,
        num_swdge_queues=2,
    )

    gcw = nc.dram_tensor("gcw", [M_SH, K], F32, kind="ExternalInput").ap()
    aa = nc.dram_tensor("aa", [M_SH, K], F32, kind="ExternalInput").ap()
    gcb = nc.dram_tensor("gcb", [M_SH, DH], F32, kind="ExternalInput").ap()
    h = nc.dram_tensor("h", [B_SH, NC_, NT, DH], F32, kind="ExternalInput").ap()
    out = nc.dram_tensor("out", [B_SH, NS, T_SH, DH], F32, kind="ExternalOutput").ap()

    # row-permuted views: m' = s*T_SH + t  (s-major)
    gcw_p = gcw.rearrange("(t s) k -> s t k", t=T_SH)
    aa_p = aa.rearrange("(t s) k -> s t k", t=T_SH)
    gcb_p = gcb.rearrange("(t s) d -> s t d", t=T_SH)

    with tile.TileContext(nc) as tc:
        with ExitStack() as ctx:
            ident_pool = ctx.enter_context(tc.tile_pool(name="ident", bufs=1))
            x_pool = ctx.enter_context(tc.tile_pool(name="x", bufs=KT))
            gw_pool = ctx.enter_context(tc.tile_pool(name="gw", bufs=4))
            aa_pool = ctx.enter_context(tc.tile_pool(name="aam", bufs=4))
            am_pool = ctx.enter_context(tc.tile_pool(name="am", bufs=2))
            at_pool = ctx.enter_context(tc.tile_pool(name="at", bufs=2))
            gcb_pool = ctx.enter_context(tc.tile_pool(name="gcb", bufs=MT))
            out_pool = ctx.enter_context(tc.tile_pool(name="out", bufs=2))
            ptr_pool = ctx.enter_context(
                tc.tile_pool(name="ptr", bufs=2, space="PSUM")
            )
            pmm_pool = ctx.enter_context(
                tc.tile_pool(name="pmm", bufs=2, space="PSUM")
            )

            ident = ident_pool.tile([128, 128], BF16)
            masks.make_identity(nc, ident[:])

            # Interleave the A-stream prefetch (per-m-tile critical path
            # feeder) with the resident X tiles so neither starves: queue
            # order on the SWDGE ring follows program order.
            gw_tiles, aa_tiles, gcb_tiles, x_tiles = [], [], [], []
            for mt in range(MT):
                srows = slice(S_PT * mt, S_PT * (mt + 1))
                gw_t = gw_pool.tile([128, K], BF16)
                nc.gpsimd.dma_start(out=gw_t[:], in_=gcw_p[srows])
                aa_t = aa_pool.tile([128, K], BF16)
                nc.gpsimd.dma_start(out=aa_t[:], in_=aa_p[srows])
                gw_tiles.append(gw_t)
                aa_tiles.append(aa_t)
                # X[k'-tile] = [128 (c,t), 1024 (b,d)], cast f32->bf16 in
                # the SWDGE DMA datapath; 4 per m-tile covers all 32.
                for j in range(4 * mt, 4 * mt + 4):
                    xt = x_pool.tile([128, NFREE], BF16)
                    src = h[:, 2 * j : 2 * j + 2, :, :].rearrange(
                        "b c t d -> (c t) b d"
                    )
                    nc.gpsimd.dma_start(out=xt[:], in_=src)
                    x_tiles.append(xt)
                if mt == 0:
                    for mt2 in range(MT):
                        srows2 = slice(S_PT * mt2, S_PT * (mt2 + 1))
                        gcb_t = gcb_pool.tile([128, DH], F32)
                        nc.sync.dma_start(out=gcb_t[:], in_=gcb_p[srows2])
                        gcb_tiles.append(gcb_t)

            for mt in range(MT):
                gw_t, aa_t = gw_tiles[mt], aa_tiles[mt]
                # masked weights with fused relu: since aa >= 0,
                # relu(gw*aa) == max(gw,0)*aa. The output AP permutes the
                # free dim from t-major k to c-major k' so the transpose and
                # matmul reads stay dense:
                # am_t[m, c*Nt + t] = max(gw[m, t*Nc+c], 0) * aa[m, t*Nc+c].
                am_t = am_pool.tile([128, K], BF16)
                nc.vector.scalar_tensor_tensor(
                    am_t[:].rearrange("m (c t) -> m t c", c=NC_),
                    gw_t[:].rearrange("m (t c) -> m t c", c=NC_),
                    0.0,
                    aa_t[:].rearrange("m (t c) -> m t c", c=NC_),
                    mybir.AluOpType.max,
                    mybir.AluOpType.mult,
                )

                # A^T for this m'-tile: 32 side-by-side [128 k', 128 m'] tiles.
                at_t = at_pool.tile([128, K], BF16)
                for g in range(KT // 8):
                    ptr = ptr_pool.tile([128, 1024], BF16)
                    for j8 in range(8):
                        j = 8 * g + j8
                        nc.tensor.transpose(
                            ptr[:, 128 * j8 : 128 * j8 + 128],
                            am_t[:, 128 * j : 128 * j + 128],
                            ident[:],
                        )
                    dstslice = at_t[:, 1024 * g : 1024 * g + 1024]
                    if g % 2 == 0:
                        nc.scalar.copy(dstslice, ptr[:])
                    else:
                        nc.vector.tensor_copy(dstslice, ptr[:])

                # 32 accumulating matmuls: psum[m', (b,d)] += A^T[k']^T @ X[k']
                pm = pmm_pool.tile([128, NFREE], F32)
                for j in range(KT):
                    for nh in range(NFREE // 512):
                        nc.tensor.matmul(
                            pm[:, 512 * nh : 512 * nh + 512],
                            at_t[:, 128 * j : 128 * j + 128],
                            x_tiles[j][:, 512 * nh : 512 * nh + 512],
                            start=(j == 0),
                            stop=(j == KT - 1),
                        )

                # epilogue: bias add (broadcast over b) + relu, then store
                o_t = out_pool.tile([128, NFREE], F32)
                bias = gcb_tiles[mt][:].unsqueeze(1).broadcast_to(
                    (128, B_SH, DH)
                )
                nc.vector.tensor_add(
                    o_t[:].rearrange("p (b d) -> p b d", b=B_SH),
                    pm[:].rearrange("p (b d) -> p b d", b=B_SH),
                    bias,
                )
                nc.scalar.activation(
                    o_t[:], o_t[:], mybir.ActivationFunctionType.Relu
                )

                srows = slice(S_PT * mt, S_PT * (mt + 1))
                dst = out[:, srows, :, :].rearrange("b s t d -> (s t) b d")
                nc.sync.dma_start(out=dst, in_=o_t[:])

    nc.compile()
    return nc


def _build_compact():
    """Variant for the (expected) tiled AA_mask: mask[m, k] depends only on
    (m % Ns, k % Nc), so each core loads a tiny per-m-tile [128, Nc] mask
    instead of the full 16.8MB shard -- per-core HBM reads drop ~33%.

    Schedule: a "triangle" of the first 3 m-tiles accumulates both batch
    halves against X tiles as they stream in (6 one-bank PSUM accumulators
    + 2 transpose-staging banks = all of PSUM), so the in-order PE stream
    has matmul work throughout the h/gcw stream. The remaining 5 m-tiles
    run as a PE-bound sequential pipeline fed by trailing gcw loads, which
    have large arrival slack by then."""
    nc = bacc.Bacc(
        "TRN2",
        target_bir_lowering=False,
        debug=False,
        enable_asserts=False,
        num_devices=8,
        num_swdge_queues=2,
    )

    gcw = nc.dram_tensor("gcw", [M_SH, K], F32, kind="ExternalInput").ap()
    msk = nc.dram_tensor("msk", [128, MT * NC_], F32, kind="ExternalInput").ap()
    gcb = nc.dram_tensor("gcb", [M_SH, DH], F32, kind="ExternalInput").ap()
    h = nc.dram_tensor("h", [B_SH, NC_, NT, DH], F32, kind="ExternalInput").ap()
    out = nc.dram_tensor("out", [B_SH, NS, T_SH, DH], F32, kind="ExternalOutput").ap()

    gcw_p = gcw.rearrange("(t s) k -> s t k", t=T_SH)
    gcb_p = gcb.rearrange("(t s) d -> s t d", t=T_SH)

    NTRI = 4  # m-tiles in the streaming triangle (both batch halves)

    with tile.TileContext(nc) as tc:
        with ExitStack() as ctx:
            ident_pool = ctx.enter_context(tc.tile_pool(name="ident", bufs=1))
            x_pool = ctx.enter_context(tc.tile_pool(name="x", bufs=KT))
            gw_pool = ctx.enter_context(tc.tile_pool(name="gw", bufs=4))
            msk_pool = ctx.enter_context(tc.tile_pool(name="msk", bufs=1))
            am_pool = ctx.enter_context(tc.tile_pool(name="am", bufs=2))
            at_pool = ctx.enter_context(tc.tile_pool(name="at", bufs=20))
            gcb_pool = ctx.enter_context(tc.tile_pool(name="gcb", bufs=MT))
            out_pool = ctx.enter_context(tc.tile_pool(name="out", bufs=4))
            ps_pool = ctx.enter_context(
                tc.tile_pool(name="ps", bufs=8, space="PSUM")
            )

            ident = ident_pool.tile([128, 128], BF16)
            masks.make_identity(nc, ident[:])

            gcb_tiles, gw_tiles, x_tiles, at_tiles = [], [], [], {}
            pms = {}

            msk_f32 = msk_pool.tile([128, MT * NC_], F32)
            nc.sync.dma_start(out=msk_f32[:], in_=msk)
            msk_all = msk_pool.tile([128, MT * NC_], BF16)
            nc.vector.tensor_copy(msk_all[:], msk_f32[:])
            msk_tiles = [
                msk_all[:, NC_ * i : NC_ * (i + 1)] for i in range(MT)
            ]

            def emit_gw_dma(mt):
                srows = slice(S_PT * mt, S_PT * (mt + 1))
                gw_t = gw_pool.tile([128, K], BF16, tag="gw", name=f"gw_{mt}")
                nc.gpsimd.dma_start(out=gw_t[:], in_=gcw_p[srows])
                gw_tiles.append(gw_t)

            def emit_x_dmas(r):
                for j in range(4 * r, 4 * r + 4):
                    xt = x_pool.tile([128, NFREE], BF16, tag="x", name=f"x_{j}")
                    src = h[:, 2 * j : 2 * j + 2, :, :].rearrange(
                        "b c t d -> (c t) b d"
                    )
                    nc.gpsimd.dma_start(out=xt[:], in_=src)
                    x_tiles.append(xt)

            def emit_prep(mt):
                am_t = am_pool.tile([128, K], BF16, tag="am", name=f"am_{mt}")
                at_q = [
                    at_pool.tile([128, K // 4], BF16, tag="at", name=f"at_{mt}_{q}")
                    for q in range(4)
                ]
                # am[m, c*Nt+t] = max(gw[m, t*Nc+c], 0) * mask[m, c], in
                # c-quarters so transposes start after 1/4 of the DVE work
                for ch in range(4):
                    cs = slice(NC_ // 4 * ch, NC_ // 4 * (ch + 1))
                    ks = slice(K // 4 * ch, K // 4 * (ch + 1))
                    nc.vector.scalar_tensor_tensor(
                        am_t[:, ks].rearrange("m (c t) -> m t c", c=NC_ // 4),
                        gw_tiles[mt][:].rearrange("m (t c) -> m t c", c=NC_)[
                            :, :, cs
                        ],
                        0.0,
                        msk_tiles[mt][:, cs].unsqueeze(1).broadcast_to(
                            (128, NT, NC_ // 4)
                        ),
                        mybir.AluOpType.max,
                        mybir.AluOpType.mult,
                    )
                    for g in range(ch, ch + 1):
                        ptr = ps_pool.tile(
                            [128, 1024], BF16, tag="ps", name=f"ptr_{g}"
                        )
                        for j8 in range(8):
                            j = 8 * g + j8
                            nc.tensor.transpose(
                                ptr[:, 128 * j8 : 128 * j8 + 128],
                                am_t[:, 128 * j : 128 * j + 128],
                                ident[:],
                            )
                        dstslice = at_q[g][:]
                        if g % 2 == 0:
                            nc.scalar.copy(dstslice, ptr[:])
                        else:
                            nc.vector.tensor_copy(dstslice, ptr[:])
                at_tiles[mt] = at_q

            def emit_mms(mt, ks, bh):
                pm = pms[(mt, bh)]
                at_q = at_tiles[mt]
                for k in ks:
                    q, kq = k // 8, k % 8
                    nc.tensor.matmul(
                        pm[:],
                        at_q[q][:, 128 * kq : 128 * kq + 128],
                        x_tiles[k][:, 512 * bh : 512 * bh + 512],
                        start=(k == 0),
                        stop=(k == KT - 1),
                    )

            def emit_epi(mt, bh):
                pm = pms.pop((mt, bh))
                o_t = out_pool.tile([128, 512], F32, tag="out", name=f"o_{mt}_{bh}")
                bias = gcb_tiles[mt][:].unsqueeze(1).broadcast_to(
                    (128, 4, DH)
                )
                nc.vector.tensor_add(
                    o_t[:].rearrange("p (b d) -> p b d", b=4),
                    pm[:].rearrange("p (b d) -> p b d", b=4),
                    bias,
                )
                nc.scalar.activation(
                    o_t[:], o_t[:], mybir.ActivationFunctionType.Relu
                )
                srows = slice(S_PT * mt, S_PT * (mt + 1))
                dst = out[4 * bh : 4 * bh + 4, srows, :, :].rearrange(
                    "b s t d -> (s t) b d"
                )
                nc.sync.dma_start(out=dst, in_=o_t[:])

            def alloc_pm(mt, bh):
                pms[(mt, bh)] = ps_pool.tile(
                    [128, 512], F32, tag="ps", name=f"pm_{mt}_{bh}"
                )

            # ---- DMA + compute emission ----
            # streaming phase: gcw(0..2) early, X windows, triangle MMs
            for r in range(MT):
                if r < NTRI:
                    emit_gw_dma(r)
                if r >= 6 and NTRI + (r - 6) < MT:
                    emit_gw_dma(NTRI + (r - 6))  # early trailing gcw
                emit_x_dmas(r)
                if r == 2:
                    for i in range(MT):
                        srows2 = slice(S_PT * i, S_PT * (i + 1))
                        gcb_t = gcb_pool.tile(
                            [128, DH], F32, tag="gcb", name=f"gcb_{i}"
                        )
                        nc.sync.dma_start(out=gcb_t[:], in_=gcb_p[srows2])
                        gcb_tiles.append(gcb_t)
                if r < NTRI:
                    if r < NTRI - 1:
                        # allocate ahead of the prep's ptr tiles so the
                        # accumulators land on distinct PSUM slots (avoids a
                        # slot WAR stalling the first catch-up matmuls)
                        alloc_pm(r, 0)
                        alloc_pm(r, 1)
                    emit_prep(r)
                for mt in range(min(r, NTRI - 1) + 1):
                    if mt == r:
                        if (mt, 0) not in pms:
                            alloc_pm(mt, 0)
                            alloc_pm(mt, 1)
                        ks = range(0, 4 * r + 4)
                    else:
                        ks = range(4 * r, 4 * r + 4)
                    for k in ks:
                        for bh in range(2):
                            emit_mms(mt, [k], bh)

            # remaining trailing gcw loads: needed only as the sequential
            # tail consumes them, well after the X stream completes
            for mt in range(NTRI + 2, MT):
                emit_gw_dma(mt)

            # triangle epilogues, then the PE-bound sequential tail
            for mt in range(NTRI):
                emit_epi(mt, 0)
                emit_epi(mt, 1)
            for mt in range(NTRI, MT):
                emit_prep(mt)
                for bh in range(2):
                    alloc_pm(mt, bh)
                    emit_mms(mt, range(KT), bh)
                    emit_epi(mt, bh)

    nc.compile()
    return nc


def _build_full_tri():
    """General-mask fallback with the same triangular schedule: streams
    the full AA shard alongside GCW (both bf16-cast in the DMA)."""
    nc = bacc.Bacc(
        "TRN2",
        target_bir_lowering=False,
        debug=False,
        enable_asserts=False,
        num_devices=8,
        num_swdge_queues=2,
    )

    gcw = nc.dram_tensor("gcw", [M_SH, K], F32, kind="ExternalInput").ap()
    aa = nc.dram_tensor("aa", [M_SH, K], F32, kind="ExternalInput").ap()
    gcb = nc.dram_tensor("gcb", [M_SH, DH], F32, kind="ExternalInput").ap()
    h = nc.dram_tensor("h", [B_SH, NC_, NT, DH], F32, kind="ExternalInput").ap()
    out = nc.dram_tensor("out", [B_SH, NS, T_SH, DH], F32, kind="ExternalOutput").ap()

    gcw_p = gcw.rearrange("(t s) k -> s t k", t=T_SH)
    aa_p = aa.rearrange("(t s) k -> s t k", t=T_SH)
    gcb_p = gcb.rearrange("(t s) d -> s t d", t=T_SH)

    NTRI = 4  # m-tiles in the streaming triangle (both batch halves)

    with tile.TileContext(nc) as tc:
        with ExitStack() as ctx:
            ident_pool = ctx.enter_context(tc.tile_pool(name="ident", bufs=1))
            x_pool = ctx.enter_context(tc.tile_pool(name="x", bufs=KT))
            gw_pool = ctx.enter_context(tc.tile_pool(name="gw", bufs=4))
            aa_pool = ctx.enter_context(tc.tile_pool(name="aam", bufs=4))
            am_pool = ctx.enter_context(tc.tile_pool(name="am", bufs=2))
            at_pool = ctx.enter_context(tc.tile_pool(name="at", bufs=20))
            gcb_pool = ctx.enter_context(tc.tile_pool(name="gcb", bufs=MT))
            out_pool = ctx.enter_context(tc.tile_pool(name="out", bufs=4))
            ps_pool = ctx.enter_context(
                tc.tile_pool(name="ps", bufs=8, space="PSUM")
            )

            ident = ident_pool.tile([128, 128], BF16)
            masks.make_identity(nc, ident[:])

            gcb_tiles, gw_tiles, x_tiles, at_tiles = [], [], [], {}
            pms = {}

            aa_tiles = []

            def emit_gw_dma(mt):
                srows = slice(S_PT * mt, S_PT * (mt + 1))
                gw_t = gw_pool.tile([128, K], BF16, tag="gw", name=f"gw_{mt}")
                nc.gpsimd.dma_start(out=gw_t[:], in_=gcw_p[srows])
                gw_tiles.append(gw_t)
                aa_t = aa_pool.tile([128, K], BF16, tag="aa", name=f"aa_{mt}")
                nc.gpsimd.dma_start(out=aa_t[:], in_=aa_p[srows])
                aa_tiles.append(aa_t)

            def emit_x_dmas(r):
                for j in range(4 * r, 4 * r + 4):
                    xt = x_pool.tile([128, NFREE], BF16, tag="x", name=f"x_{j}")
                    src = h[:, 2 * j : 2 * j + 2, :, :].rearrange(
                        "b c t d -> (c t) b d"
                    )
                    nc.gpsimd.dma_start(out=xt[:], in_=src)
                    x_tiles.append(xt)

            def emit_prep(mt):
                am_t = am_pool.tile([128, K], BF16, tag="am", name=f"am_{mt}")
                at_q = [
                    at_pool.tile([128, K // 4], BF16, tag="at", name=f"at_{mt}_{q}")
                    for q in range(4)
                ]
                # am[m, c*Nt+t] = max(gw[m, t*Nc+c], 0) * mask[m, c], in
                # c-quarters so transposes start after 1/4 of the DVE work
                for ch in range(4):
                    cs = slice(NC_ // 4 * ch, NC_ // 4 * (ch + 1))
                    ks = slice(K // 4 * ch, K // 4 * (ch + 1))
                    nc.vector.scalar_tensor_tensor(
                        am_t[:, ks].rearrange("m (c t) -> m t c", c=NC_ // 4),
                        gw_tiles[mt][:].rearrange("m (t c) -> m t c", c=NC_)[
                            :, :, cs
                        ],
                        0.0,
                        aa_tiles[mt][:].rearrange(
                            "m (t c) -> m t c", c=NC_
                        )[:, :, cs],
                        mybir.AluOpType.max,
                        mybir.AluOpType.mult,
                    )
                    for g in range(ch, ch + 1):
                        ptr = ps_pool.tile(
                            [128, 1024], BF16, tag="ps", name=f"ptr_{g}"
                        )
                        for j8 in range(8):
                            j = 8 * g + j8
                            nc.tensor.transpose(
                                ptr[:, 128 * j8 : 128 * j8 + 128],
                                am_t[:, 128 * j : 128 * j + 128],
                                ident[:],
                            )
                        dstslice = at_q[g][:]
                        if g % 2 == 0:
                            nc.scalar.copy(dstslice, ptr[:])
                        else:
                            nc.vector.tensor_copy(dstslice, ptr[:])
                at_tiles[mt] = at_q

            def emit_mms(mt, ks, bh):
                pm = pms[(mt, bh)]
                at_q = at_tiles[mt]
                for k in ks:
                    q, kq = k // 8, k % 8
                    nc.tensor.matmul(
                        pm[:],
                        at_q[q][:, 128 * kq : 128 * kq + 128],
                        x_tiles[k][:, 512 * bh : 512 * bh + 512],
                        start=(k == 0),
                        stop=(k == KT - 1),
                    )

            def emit_epi(mt, bh):
                pm = pms.pop((mt, bh))
                o_t = out_pool.tile([128, 512], F32, tag="out", name=f"o_{mt}_{bh}")
                bias = gcb_tiles[mt][:].unsqueeze(1).broadcast_to(
                    (128, 4, DH)
                )
                nc.vector.tensor_add(
                    o_t[:].rearrange("p (b d) -> p b d", b=4),
                    pm[:].rearrange("p (b d) -> p b d", b=4),
                    bias,
                )
                nc.scalar.activation(
                    o_t[:], o_t[:], mybir.ActivationFunctionType.Relu
                )
                srows = slice(S_PT * mt, S_PT * (mt + 1))
                dst = out[4 * bh : 4 * bh + 4, srows, :, :].rearrange(
                    "b s t d -> (s t) b d"
                )
                nc.sync.dma_start(out=dst, in_=o_t[:])

            def alloc_pm(mt, bh):
                pms[(mt, bh)] = ps_pool.tile(
                    [128, 512], F32, tag="ps", name=f"pm_{mt}_{bh}"
                )

            # ---- DMA + compute emission ----
            # streaming phase: gcw(0..2) early, X windows, triangle MMs
            for r in range(MT):
                if r < NTRI:
                    emit_gw_dma(r)
                if r >= 6 and NTRI + (r - 6) < MT:
                    emit_gw_dma(NTRI + (r - 6))  # early trailing gcw
                emit_x_dmas(r)
                if r == 2:
                    for i in range(MT):
                        srows2 = slice(S_PT * i, S_PT * (i + 1))
                        gcb_t = gcb_pool.tile(
                            [128, DH], F32, tag="gcb", name=f"gcb_{i}"
                        )
                        nc.sync.dma_start(out=gcb_t[:], in_=gcb_p[srows2])
                        gcb_tiles.append(gcb_t)
                if r < NTRI:
                    if r < NTRI - 1:
                        # allocate ahead of the prep's ptr tiles so the
                        # accumulators land on distinct PSUM slots (avoids a
                        # slot WAR stalling the first catch-up matmuls)
                        alloc_pm(r, 0)
                        alloc_pm(r, 1)
                    emit_prep(r)
                for mt in range(min(r, NTRI - 1) + 1):
                    if mt == r:
                        if (mt, 0) not in pms:
                            alloc_pm(mt, 0)
                            alloc_pm(mt, 1)
                        ks = range(0, 4 * r + 4)
                    else:
                        ks = range(4 * r, 4 * r + 4)
                    for k in ks:
                        for bh in range(2):
                            emit_mms(mt, [k], bh)

            # remaining trailing gcw loads: needed only as the sequential
            # tail consumes them, well after the X stream completes
            for mt in range(NTRI + 2, MT):
                emit_gw_dma(mt)

            # triangle epilogues, then the PE-bound sequential tail
            for mt in range(NTRI):
                emit_epi(mt, 0)
                emit_epi(mt, 1)
            for mt in range(NTRI, MT):
                emit_prep(mt)
                for bh in range(2):
                    alloc_pm(mt, bh)
                    emit_mms(mt, range(KT), bh)
                    emit_epi(mt, bh)

    nc.compile()
    return nc




def _mask_small(AA_mask):
    """[128, MT*Nc] per-m'-tile mask rows, mt-major along the free dim
    (identical for every core)."""
    A64 = AA_mask[:NS, :NC_]
    ms = np.empty((128, MT * NC_), dtype=np.float32)
    for mt in range(MT):
        for p in range(128):
            s = S_PT * mt + p // T_SH
            ms[p, NC_ * mt : NC_ * (mt + 1)] = A64[s]
    return ms


def _is_tiled(AA_mask):
    A64 = AA_mask[:NS, :NC_]
    return np.array_equal(AA_mask, np.tile(A64, (NT, NT)))


def _make_in_maps(h, AA_mask, GCW, GCB, compact):
    in_maps = []
    ms = _mask_small(AA_mask) if compact else None
    for r in range(8):
        rq, bq = r % P_ROW, r // P_ROW
        rs = slice(M_SH * rq, M_SH * (rq + 1))
        bs_ = slice(B_SH * bq, B_SH * (bq + 1))
        m = {
            "gcw": np.ascontiguousarray(GCW[rs], np.float32),
            "gcb": np.ascontiguousarray(GCB[rs], np.float32),
            "h": np.ascontiguousarray(h[bs_], np.float32),
        }
        if compact:
            m["msk"] = ms
        else:
            m["aa"] = np.ascontiguousarray(AA_mask[rs], np.float32)
        in_maps.append(m)
    return in_maps


def _assemble(results):
    full = np.empty((BS, NS, NT, DH), dtype=np.float32)
    for r in range(8):
        rq, bq = r % P_ROW, r // P_ROW
        full[
            B_SH * bq : B_SH * (bq + 1), :, T_SH * rq : T_SH * (rq + 1), :
        ] = results[r]["out"]
    return full



def kernel(h, e, AA_mask, GCW, GCB):
    h = np.asarray(h)
    AA_mask = np.asarray(AA_mask)
    GCW = np.asarray(GCW)
    GCB = np.asarray(GCB)

    if _is_tiled(AA_mask):
        try:
            A64 = AA_mask[:NS, :NC_]
            meta = _make_meta(A64)
            key = _build_key(meta)
            if key not in _cached:
                _cached[key] = _build_v2(meta)
            nc = _cached[key]
            in_maps = _host_prep(h, A64, GCW, GCB, meta)
            res = run_bass_kernel_spmd(nc, in_maps, core_ids=list(range(8)))
            return _assemble_v2_fixed(res.results, meta)
        except Exception:
            if "compact" not in _cached:
                _cached["compact"] = _build_compact()
            nc = _cached["compact"]
            in_maps = _make_in_maps(h, AA_mask, GCW, GCB, True)
            res = run_bass_kernel_spmd(nc, in_maps, core_ids=list(range(8)))
            return _assemble(res.results)

    key = "full"
    if key not in _cached:
        try:
            _cached[key] = _build_full_tri()
        except Exception:
            _cached[key] = _build()
    nc = _cached[key]
    in_maps = _make_in_maps(h, AA_mask, GCW, GCB, False)
    res = run_bass_kernel_spmd(nc, in_maps, core_ids=list(range(8)))
    return _assemble(res.results)
